# revision 8
# baseline (speedup 1.0000x reference)
"""Trainium2 Bass kernel for nn_NeuralCF (3-layer RGCN + NCF head), 8 NeuronCores.

Strategy (node-parallel over destination shards):
- Each of 8 cores owns a contiguous 12,500-node destination shard.
- Edges are sharded by dst, sorted by (dst-window, src-bank); per-edge source
  features are gathered with dma_gather (int16 indices -> 4 src banks of 25k).
- Aggregation per 128-node dst window: weighted one-hot S (built on DVE via
  iota==dstl * w) matmul'd against gathered rows -> PSUM [feat, 128dst x 2rel].
- Dense phase: (agg0|agg1|x) @ (Wrel0.T|Wrel1.T|Wroot.T) via 3 PE matmuls,
  bias + ReLU + LayerNorm in fp32, write f16 shard, AllGather across cores.
- Head: indirect-DMA gather of u/v rows, GMF + MLP on-chip, per-core 2048
  scores, concatenated on host.
"""
import math
import numpy as np

import concourse.bass as bass
import concourse.bacc as bacc
import concourse.mybir as mybir
from concourse import tile

N_NODES = 100000
N_EDGES = 1000000
EMB = 128
N_REL = 2
BATCH = 16384
HIDDEN = [256, 128, 64, 32]
LN_EPS = 1e-5

NCORES = 8
N_LOC = N_NODES // NCORES          # 12500
WIN = 128
N_WIN = math.ceil(N_LOC / WIN)     # 98 (last window 84 nodes)
BANKS = 4
BANK_SZ = 25000
G = 8                              # chunks per dma_gather / S-build batch
P = 128
N_LOC_PAD = 12512                  # N_LOC padded to multiple of 16 for DMA transpose

F16 = mybir.dt.float16
F32 = mybir.dt.float32
I16 = mybir.dt.int16
I32 = mybir.dt.int32

_cache = {}


# ---------------------------------------------------------------- host prep
def _prep_edges(src, dst, w0, w1):
    """Returns (K [N_WIN,BANKS] chunk counts, per-core streams dict)."""
    core = dst // N_LOC
    per_core = []
    for c in range(NCORES):
        m = core == c
        s = src[m]
        d = dst[m] - c * N_LOC
        ww0 = w0[m]
        ww1 = w1[m]
        win = d // WIN
        bank = s // BANK_SZ
        order = np.lexsort((bank, win))
        s, d, ww0, ww1, win, bank = (a[order] for a in (s, d, ww0, ww1, win, bank))
        # counts[w, b]
        cnt = np.zeros((N_WIN, BANKS), np.int64)
        np.add.at(cnt, (win, bank), 1)
        per_core.append(dict(s=s, d=d, w0=ww0, w1=ww1, cnt=cnt))
    cnt_max = np.maximum.reduce([pc["cnt"] for pc in per_core])
    K = (cnt_max + P - 1) // P
    K[:, 0] = np.maximum(K[:, 0], 1)  # ensure >=1 chunk per window
    return K, per_core


def _build_core_streams(K, pc):
    """Build padded per-(w,b) arrays and lay out gather/metadata streams."""
    C = int(K.sum())
    C_b = K.sum(axis=0).astype(int)          # chunks per bank
    NG_b = [(int(cb) + G - 1) // G for cb in C_b]

    s, d, w0, w1, cnt = pc["s"], pc["d"], pc["w0"], pc["w1"], pc["cnt"]
    # prefix offsets of (w,b) runs in the sorted edge arrays
    run_off = np.zeros((N_WIN, BANKS), np.int64)
    flat = cnt.reshape(-1)
    run_off.reshape(-1)[1:] = np.cumsum(flat)[:-1]

    C_pad = ((C + G - 1) // G) * G
    # per-bank gather index stream (int16 src-local), padded per (w,b) to K*128
    gidx_banks = [np.zeros(int(K[:, b].sum()) * P, np.int16) for b in range(BANKS)]
    # global-chunk-ordered metadata (padded to a multiple of G for S-build batches)
    dstl = np.zeros((C_pad, P), np.float16)
    ww01 = np.zeros((C_pad, 2, P), np.float16)

    bank_pos = [0] * BANKS   # chunk position within bank stream
    cidx = 0
    chunk_map = []           # (bank, bank_chunk_pos) per global chunk
    for w in range(N_WIN):
        for b in range(BANKS):
            k_wb = int(K[w, b])
            if k_wb == 0:
                continue
            n = int(cnt[w, b])
            o = int(run_off[w, b])
            slots = k_wb * P
            srcl = np.zeros(slots, np.int16)
            srcl[:n] = (s[o:o + n] - b * BANK_SZ).astype(np.int16)
            dl = np.zeros(slots, np.float16)
            dl[:n] = (d[o:o + n] - w * WIN).astype(np.float16)
            v0 = np.zeros(slots, np.float16)
            v0[:n] = w0[o:o + n].astype(np.float16)
            v1 = np.zeros(slots, np.float16)
            v1[:n] = w1[o:o + n].astype(np.float16)
            bp = bank_pos[b]
            gidx_banks[b][bp * P:(bp + k_wb) * P] = srcl
            for k in range(k_wb):
                dstl[cidx] = dl[k * P:(k + 1) * P]
                ww01[cidx, 0] = v0[k * P:(k + 1) * P]
                ww01[cidx, 1] = v1[k * P:(k + 1) * P]
                chunk_map.append((b, bp + k))
                cidx += 1
            bank_pos[b] += k_wb
    assert cidx == C

    # wrap each bank's index stream for dma_gather: per gather of G*128 rows,
    # idx j -> partition j%16 (replicated x8), col j//16
    gidx_cols = []
    for b in range(BANKS):
        arr = gidx_banks[b]
        padded = np.zeros(NG_b[b] * G * P, np.int16)
        padded[:arr.size] = arr
        blocks = []
        for g in range(NG_b[b]):
            blk = padded[g * G * P:(g + 1) * G * P]
            blocks.append(np.tile(blk.reshape(G * P // 16, 16).T, (8, 1)))
        gidx_cols.append(np.concatenate(blocks, axis=1) if blocks
                         else np.zeros((P, 0), np.int16))
    gidx = np.concatenate(gidx_cols, axis=1)

    return dict(
        gidx=gidx,                                  # [128, sum(NG_b)*G*8]
        dstl=np.ascontiguousarray(dstl.T),          # [128, C_pad]
        w01=np.ascontiguousarray(
            ww01.transpose(2, 0, 1).reshape(P, 2 * C_pad)),  # [128, 2*C_pad]
        chunk_map=chunk_map, NG_b=NG_b, C=C, C_pad=C_pad,
    )


# ---------------------------------------------------------------- bass build
def _build_program(K, NG_b, C):
    C_pad = ((C + G - 1) // G) * G
    GI = sum(NG_b) * G * 8     # gidx cols
    goff = np.concatenate([[0], np.cumsum([n * G * 8 for n in NG_b])]).astype(int)

    nc = bacc.Bacc(None)
    x0f = nc.dram_tensor("x0f", [N_NODES, EMB], F16, kind="ExternalInput")
    x0loc = nc.dram_tensor("x0loc", [N_LOC_PAD, EMB], F16, kind="ExternalInput")
    gidx = nc.dram_tensor("gidx", [P, GI], I16, kind="ExternalInput")
    dstl = nc.dram_tensor("dstl", [P, C_pad], F16, kind="ExternalInput")
    w01 = nc.dram_tensor("w01", [P, 2 * C_pad], F16, kind="ExternalInput")
    iota2 = nc.dram_tensor("iota2", [P, 256], F16, kind="ExternalInput")
    wd = nc.dram_tensor("wd", [P, 9 * 128], F16, kind="ExternalInput")
    biasw = nc.dram_tensor("biasw", [P, 3 * 128], F16, kind="ExternalInput")
    lnp = nc.dram_tensor("lnp", [P, 4 * 128], F32, kind="ExternalInput")
    mlpw = nc.dram_tensor("mlpw", [P, 864], F16, kind="ExternalInput")
    mlpb = nc.dram_tensor("mlpb", [P, 5], F32, kind="ExternalInput")
    oww = nc.dram_tensor("oww", [P, 160], F16, kind="ExternalInput")
    uvidx = nc.dram_tensor("uvidx", [P, 32], I32, kind="ExternalInput")
    score = nc.dram_tensor("score", [BATCH // NCORES], F32, kind="ExternalOutput")

    xloc = [nc.dram_tensor(f"x{l}loc", [N_LOC_PAD, EMB], F16) for l in (1, 2, 3)]
    xfull = [nc.dram_tensor(f"x{l}f", [N_NODES, EMB], F16, addr_space="Shared")
             for l in (1, 2, 3)]

    AX = mybir.AxisListType
    OP = mybir.AluOpType
    AF = mybir.ActivationFunctionType
    from concourse.masks import make_identity

    with tile.TileContext(nc) as tc:
        with tc.tile_pool(name="const", bufs=1) as cp, \
             tc.tile_pool(name="xt", bufs=2) as xtp, \
             tc.tile_pool(name="stg", bufs=3) as stgp, \
             tc.tile_pool(name="sbl", bufs=2) as sbp, \
             tc.tile_pool(name="win", bufs=3) as wp, \
             tc.tile_pool(name="head", bufs=1) as hp, \
             tc.tile_pool(name="psA", bufs=2, space="PSUM") as psA, \
             tc.tile_pool(name="psB", bufs=2, space="PSUM") as psB, \
             tc.tile_pool(name="psT", bufs=1, space="PSUM") as psT:

            # ---- persistent loads
            gidx_sb = cp.tile([P, GI], I16)
            nc.sync.dma_start(out=gidx_sb[:], in_=gidx[:])
            dstl_sb = cp.tile([P, C_pad], F16)
            nc.sync.dma_start(out=dstl_sb[:], in_=dstl[:])
            w01_sb = cp.tile([P, 2 * C_pad], F16)
            nc.sync.dma_start(out=w01_sb[:], in_=w01[:])
            iota2_sb = cp.tile([P, 256], F16)
            nc.sync.dma_start(out=iota2_sb[:], in_=iota2[:])
            wd_sb = cp.tile([P, 9 * 128], F16)
            nc.sync.dma_start(out=wd_sb[:], in_=wd[:])
            biasw_sb = cp.tile([P, 3 * 128], F16)
            nc.sync.dma_start(out=biasw_sb[:], in_=biasw[:])
            lnp_sb = cp.tile([P, 4 * 128], F32)
            nc.sync.dma_start(out=lnp_sb[:], in_=lnp[:])
            mlpw_sb = cp.tile([P, 864], F16)
            nc.sync.dma_start(out=mlpw_sb[:], in_=mlpw[:])
            mlpb_sb = cp.tile([P, 5], F32)
            nc.sync.dma_start(out=mlpb_sb[:], in_=mlpb[:])
            oww_sb = cp.tile([P, 160], F16)
            nc.sync.dma_start(out=oww_sb[:], in_=oww[:])
            uvidx_sb = cp.tile([P, 32], I32)
            nc.sync.dma_start(out=uvidx_sb[:], in_=uvidx[:])
            ident = cp.tile([P, P], F16)
            make_identity(nc, ident[:])

            Kl = K.tolist()
            for l in range(3):
                xsf = [x0f, xfull[0], xfull[1]][l]
                xsl = [x0loc, xloc[0], xloc[1]][l]
                xT = xtp.tile([P, N_LOC_PAD], F16, tag="xT")
                nc.sync.dma_start(out=xT[:], in_=xsl[:], transpose=True)

                jb = [0] * BANKS
                cur = [None] * BANKS
                cur_s01 = None
                cidx = 0
                for wi in range(N_WIN):
                    nw = WIN if wi < N_WIN - 1 else N_LOC - WIN * (N_WIN - 1)
                    nch_w = sum(Kl[wi])
                    pw = psA.tile([P, 256], F32, tag="pw")
                    ci_w = 0
                    for b in range(BANKS):
                        for k in range(Kl[wi][b]):
                            if jb[b] % G == 0:
                                cur[b] = stgp.tile([P, G * 128], F16, tag=f"stg{b}", name=f"stg{b}")
                                g = jb[b] // G
                                nc.gpsimd.dma_gather(
                                    out_ap=cur[b][:].rearrange(
                                        "p (g d) -> p g d", d=128),
                                    in_ap=xsf[b * BANK_SZ:(b + 1) * BANK_SZ, :],
                                    idxs_ap=gidx_sb[:, goff[b] + g * G * 8:
                                                    goff[b] + (g + 1) * G * 8],
                                    num_idxs=G * 128, num_idxs_reg=G * 128,
                                    elem_size=EMB)
                            if cidx % G == 0:
                                eq2 = sbp.tile([P, G * 256], F16, tag="eq2")
                                s01 = sbp.tile([P, G * 256], F16, tag="s01")
                                c0 = cidx
                                nc.vector.tensor_tensor(
                                    out=eq2[:],
                                    in0=bass.AP(iota2_sb[:].tensor,
                                                iota2_sb[:].offset,
                                                [iota2_sb[:].ap[0], [0, G], [1, 256]]),
                                    in1=bass.AP(dstl_sb[:].tensor,
                                                dstl_sb[:, c0:c0 + G].offset,
                                                [dstl_sb[:].ap[0], [1, G], [0, 256]]),
                                    op=OP.is_equal)
                                nc.vector.tensor_tensor(
                                    out=s01[:],
                                    in0=eq2[:],
                                    in1=bass.AP(w01_sb[:].tensor,
                                                w01_sb[:, 2 * c0:2 * (c0 + G)].offset,
                                                [w01_sb[:].ap[0], [2, G], [1, 2],
                                                 [0, 128]]),
                                    op=OP.mult)
                                cur_s01 = s01
                            blk = cur[b][:, (jb[b] % G) * 128:(jb[b] % G + 1) * 128]
                            nc.tensor.matmul(
                                pw[:, :],
                                lhsT=blk,
                                rhs=cur_s01[:, (cidx % G) * 256:(cidx % G + 1) * 256],
                                start=(ci_w == 0), stop=(ci_w == nch_w - 1))
                            jb[b] += 1
                            cidx += 1
                            ci_w += 1
                    # dense phase for window wi
                    a01 = wp.tile([P, 256], F16, tag="a01")
                    nc.scalar.copy(out=a01[:], in_=pw[:])
                    px = psB.tile([P, 128], F32, tag="px")
                    base = l * 384
                    nc.tensor.matmul(px[:nw, :], lhsT=a01[:, 0:nw],
                                     rhs=wd_sb[:, base:base + 128],
                                     start=True, stop=False)
                    nc.tensor.matmul(px[:nw, :], lhsT=a01[:, 128:128 + nw],
                                     rhs=wd_sb[:, base + 128:base + 256],
                                     start=False, stop=False)
                    nc.tensor.matmul(px[:nw, :],
                                     lhsT=xT[:, wi * WIN:wi * WIN + nw],
                                     rhs=wd_sb[:, base + 256:base + 384],
                                     start=False, stop=True)
                    xn = wp.tile([P, 128], F32, tag="xn")
                    nc.vector.tensor_tensor(
                        out=xn[:nw, :], in0=px[:nw, :],
                        in1=biasw_sb[:nw, l * 128:(l + 1) * 128], op=OP.add)
                    xo = wp.tile([P, 128], F16, tag="xo")
                    if l < 2:
                        nc.scalar.activation(out=xn[:nw, :], in_=xn[:nw, :],
                                             func=AF.Relu)
                        mu = wp.tile([P, 1], F32, tag="mu")
                        nc.vector.tensor_reduce(out=mu[:nw], in_=xn[:nw, :],
                                                axis=AX.X, op=OP.add)
                        nc.vector.tensor_scalar(out=mu[:nw], in0=mu[:nw],
                                                scalar1=1.0 / EMB, scalar2=None,
                                                op0=OP.mult)
                        cen = wp.tile([P, 128], F32, tag="cen")
                        nc.vector.tensor_scalar(out=cen[:nw, :], in0=xn[:nw, :],
                                                scalar1=mu[:nw], scalar2=None,
                                                op0=OP.subtract)
                        sq = wp.tile([P, 128], F32, tag="sq")
                        nc.vector.tensor_tensor(out=sq[:nw, :], in0=cen[:nw, :],
                                                in1=cen[:nw, :], op=OP.mult)
                        sv = wp.tile([P, 1], F32, tag="sv")
                        nc.vector.tensor_reduce(out=sv[:nw], in_=sq[:nw, :],
                                                axis=AX.X, op=OP.add)
                        rstd = wp.tile([P, 1], F32, tag="rstd")
                        nc.vector.tensor_scalar(out=rstd[:nw], in0=sv[:nw],
                                                scalar1=1.0 / EMB, scalar2=LN_EPS,
                                                op0=OP.mult, op1=OP.add)
                        nc.scalar.activation(out=rstd[:nw], in_=rstd[:nw],
                                             func=AF.Sqrt)
                        nc.vector.reciprocal(out=rstd[:nw], in_=rstd[:nw])
                        nc.vector.scalar_tensor_tensor(
                            out=xo[:nw, :], in0=cen[:nw, :], scalar=rstd[:nw],
                            in1=lnp_sb[:nw, l * 256:l * 256 + 128],
                            op0=OP.mult, op1=OP.mult)
                        nc.vector.tensor_tensor(
                            out=xo[:nw, :], in0=xo[:nw, :],
                            in1=lnp_sb[:nw, l * 256 + 128:l * 256 + 256],
                            op=OP.add)
                    else:
                        nc.vector.tensor_copy(out=xo[:nw, :], in_=xn[:nw, :])
                    nc.sync.dma_start(out=xloc[l][wi * WIN:wi * WIN + nw, :],
                                      in_=xo[:nw, :])
                nc.gpsimd.collective_compute(
                    "AllGather", OP.bypass,
                    replica_groups=[list(range(NCORES))],
                    ins=[xloc[l][0:N_LOC, :]], outs=[xfull[l][:]])

            # ------------- head: 2048 items per core
            NB = (BATCH // NCORES) // P    # 16 blocks
            unm = hp.tile([P, NB * 128], F16)
            vnm = hp.tile([P, NB * 128], F16)
            for k in range(NB):
                nc.gpsimd.indirect_dma_start(
                    out=unm[:, k * 128:(k + 1) * 128], out_offset=None,
                    in_=xfull[2][:, :],
                    in_offset=bass.IndirectOffsetOnAxis(
                        ap=uvidx_sb[:, k:k + 1], axis=0))
                nc.gpsimd.indirect_dma_start(
                    out=vnm[:, k * 128:(k + 1) * 128], out_offset=None,
                    in_=xfull[2][:, :],
                    in_offset=bass.IndirectOffsetOnAxis(
                        ap=uvidx_sb[:, NB + k:NB + k + 1], axis=0))
            uu = hp.tile([P, NB * 128], F32)
            nc.vector.tensor_tensor(out=uu[:], in0=unm[:], in1=unm[:], op=OP.mult)
            vv = hp.tile([P, NB * 128], F32)
            nc.vector.tensor_tensor(out=vv[:], in0=vnm[:], in1=vnm[:], op=OP.mult)
            uv = hp.tile([P, NB * 128], F32)
            nc.vector.tensor_tensor(out=uv[:], in0=unm[:], in1=vnm[:], op=OP.mult)
            nu = hp.tile([P, NB], F32)
            nc.vector.tensor_reduce(
                out=nu[:], in_=uu[:].rearrange("p (b d) -> p b d", d=128),
                axis=AX.X, op=OP.add)
            nv = hp.tile([P, NB], F32)
            nc.vector.tensor_reduce(
                out=nv[:], in_=vv[:].rearrange("p (b d) -> p b d", d=128),
                axis=AX.X, op=OP.add)
            rsu = hp.tile([P, NB], F32)
            nc.scalar.activation(out=rsu[:], in_=nu[:], func=AF.Sqrt)
            nc.vector.reciprocal(out=rsu[:], in_=rsu[:])
            rsv = hp.tile([P, NB], F32)
            nc.scalar.activation(out=rsv[:], in_=nv[:], func=AF.Sqrt)
            nc.vector.reciprocal(out=rsv[:], in_=rsv[:])
            gmf = hp.tile([P, NB * 128], F32)
            for k in range(NB):
                nc.vector.tensor_scalar(
                    out=gmf[:, k * 128:(k + 1) * 128],
                    in0=uv[:, k * 128:(k + 1) * 128],
                    scalar1=rsu[:, k:k + 1], scalar2=rsv[:, k:k + 1],
                    op0=OP.mult, op1=OP.mult)
            gow = hp.tile([P, NB * 128], F32)
            nc.vector.tensor_tensor(
                out=gow[:],
                in0=gmf[:],
                in1=bass.AP(oww_sb[:].tensor, oww_sb[:, 0:128].offset,
                            [oww_sb[:].ap[0], [0, NB], [1, 128]]),
                op=OP.mult)
            s1 = hp.tile([P, NB], F32)
            nc.vector.tensor_reduce(
                out=s1[:], in_=gow[:].rearrange("p (b d) -> p b d", d=128),
                axis=AX.X, op=OP.add)

            # transposes for MLP
            uT = hp.tile([P, NB * 128], F16)
            vT = hp.tile([P, NB * 128], F16)
            for k in range(NB):
                pt = psT.tile([P, 128], F16, tag="pt")
                nc.tensor.transpose(out=pt[:], in_=unm[:, k * 128:(k + 1) * 128],
                                    identity=ident[:])
                nc.scalar.copy(out=uT[:, k * 128:(k + 1) * 128], in_=pt[:])
                pt2 = psT.tile([P, 128], F16, tag="pt")
                nc.tensor.transpose(out=pt2[:], in_=vnm[:, k * 128:(k + 1) * 128],
                                    identity=ident[:])
                nc.scalar.copy(out=vT[:, k * 128:(k + 1) * 128], in_=pt2[:])

            NI = NB * 128           # 2048 items
            CH = 512                # item chunk
            h1 = hp.tile([P, 2 * NI], F16)
            for half in range(2):
                for ch in range(NI // CH):
                    ph = psT.tile([P, CH], F32, tag="ph")
                    nc.tensor.matmul(ph[:, :],
                                     lhsT=mlpw_sb[:, half * 128:(half + 1) * 128],
                                     rhs=uT[:, ch * CH:(ch + 1) * CH],
                                     start=True, stop=False)
                    nc.tensor.matmul(ph[:, :],
                                     lhsT=mlpw_sb[:, (2 + half) * 128:(3 + half) * 128],
                                     rhs=vT[:, ch * CH:(ch + 1) * CH],
                                     start=False, stop=True)
                    nc.scalar.activation(
                        out=h1[:, half * NI + ch * CH:half * NI + (ch + 1) * CH],
                        in_=ph[:, :], func=AF.Relu, bias=mlpb_sb[:, half:half + 1])
            h2 = hp.tile([P, NI], F16)
            for ch in range(NI // CH):
                ph = psT.tile([P, CH], F32, tag="ph")
                nc.tensor.matmul(ph[:, :], lhsT=mlpw_sb[:, 512:640],
                                 rhs=h1[:, ch * CH:(ch + 1) * CH],
                                 start=True, stop=False)
                nc.tensor.matmul(ph[:, :], lhsT=mlpw_sb[:, 640:768],
                                 rhs=h1[:, NI + ch * CH:NI + (ch + 1) * CH],
                                 start=False, stop=True)
                nc.scalar.activation(out=h2[:, ch * CH:(ch + 1) * CH], in_=ph[:, :],
                                     func=AF.Relu, bias=mlpb_sb[:, 2:3])
            h3 = hp.tile([P, NI], F16)
            for ch in range(NI // CH):
                ph = psT.tile([P, CH], F32, tag="ph")
                nc.tensor.matmul(ph[:64, :], lhsT=mlpw_sb[:, 768:832],
                                 rhs=h2[:, ch * CH:(ch + 1) * CH],
                                 start=True, stop=True)
                nc.scalar.activation(out=h3[:64, ch * CH:(ch + 1) * CH],
                                     in_=ph[:64, :], func=AF.Relu,
                                     bias=mlpb_sb[:64, 3:4])
            h4 = hp.tile([P, NI], F16)
            for ch in range(NI // CH):
                ph = psT.tile([P, CH], F32, tag="ph")
                nc.tensor.matmul(ph[:32, :], lhsT=mlpw_sb[:64, 832:864],
                                 rhs=h3[:64, ch * CH:(ch + 1) * CH],
                                 start=True, stop=True)
                nc.scalar.activation(out=h4[:32, ch * CH:(ch + 1) * CH],
                                     in_=ph[:32, :], func=AF.Relu,
                                     bias=mlpb_sb[:32, 4:5])
            s2 = hp.tile([P, NB], F32)
            for k in range(NB):
                pt = psT.tile([P, 32], F16, tag="pt32")
                nc.tensor.transpose(out=pt[:, :], in_=h4[:32, k * 128:(k + 1) * 128],
                                    identity=ident[:32, :32])
                tmp = hp.tile([P, 32], F32, tag="htmp")
                nc.vector.tensor_tensor(out=tmp[:], in0=pt[:, :],
                                        in1=oww_sb[:, 128:160], op=OP.mult)
                nc.vector.tensor_reduce(out=s2[:, k:k + 1], in_=tmp[:],
                                        axis=AX.X, op=OP.add)
            sc = hp.tile([P, NB], F32)
            nc.vector.tensor_tensor(out=sc[:], in0=s1[:], in1=s2[:], op=OP.add)
            nc.sync.dma_start(
                out=score[:].rearrange("(b p) -> p b", p=P), in_=sc[:])

    nc.finalize()
    return nc


# ---------------------------------------------------------------- runner
def _make_runner(nc, n_cores):
    import jax
    from jax.sharding import Mesh, PartitionSpec
    from jax.experimental.shard_map import shard_map
    from concourse.bass2jax import (_bass_exec_p, install_neuronx_cc_hook,
                                    partition_id_tensor)
    install_neuronx_cc_hook()
    partition_name = nc.partition_id_tensor.name if nc.partition_id_tensor else None
    in_names, out_names, out_avals, zero_outs = [], [], [], []
    for alloc in nc.m.functions[0].allocations:
        if not isinstance(alloc, mybir.MemoryLocationSet):
            continue
        name = alloc.memorylocations[0].name
        if alloc.kind == "ExternalInput":
            if name != partition_name:
                in_names.append(name)
        elif alloc.kind == "ExternalOutput":
            shape = tuple(alloc.tensor_shape)
            dtype = mybir.dt.np(alloc.dtype)
            out_names.append(name)
            out_avals.append(jax.core.ShapedArray(shape, dtype))
            zero_outs.append(np.zeros(shape, dtype))
    n_params = len(in_names)
    n_outs = len(out_avals)
    in_names_all = in_names + out_names
    if partition_name is not None:
        in_names_all = in_names_all + [partition_name]
    donate = tuple(range(n_params, n_params + n_outs))

    def _body(*args):
        operands = list(args)
        if partition_name is not None:
            operands.append(partition_id_tensor())
        outs = _bass_exec_p.bind(
            *operands, out_avals=tuple(out_avals), in_names=tuple(in_names_all),
            out_names=tuple(out_names), lowering_input_output_aliases=(),
            sim_require_finite=True, sim_require_nnan=True, nc=nc)
        return tuple(outs)

    devices = jax.devices()[:n_cores]
    mesh = Mesh(np.asarray(devices), ("core",))
    in_specs = (PartitionSpec("core"),) * (n_params + n_outs)
    out_specs = (PartitionSpec("core"),) * n_outs
    sharded = jax.jit(
        shard_map(_body, mesh=mesh, in_specs=in_specs, out_specs=out_specs,
                  check_rep=False),
        donate_argnums=donate, keep_unused=True)

    def run(in_maps):
        per_core = [[np.asarray(m[name]) for name in in_names] for m in in_maps]
        concat_in = [np.concatenate([per_core[c][i] for c in range(n_cores)], axis=0)
                     for i in range(n_params)]
        concat_zeros = [np.zeros((n_cores * z.shape[0], *z.shape[1:]), z.dtype)
                        for z in zero_outs]
        out_arrs = sharded(*concat_in, *concat_zeros)
        return [
            {name: np.asarray(out_arrs[i]).reshape(n_cores, *out_avals[i].shape)[c]
             for i, name in enumerate(out_names)}
            for c in range(n_cores)
        ], (sharded, concat_in, zero_outs, n_cores, out_avals, out_names)

    return run


# ---------------------------------------------------------------- kernel
def kernel(user_indices, item_indices, edge_index, edge_type, edge_weight, params):
    user_indices = np.asarray(user_indices).astype(np.int64)
    item_indices = np.asarray(item_indices).astype(np.int64)
    edge_index = np.asarray(edge_index).astype(np.int64)
    edge_type = np.asarray(edge_type).astype(np.int64)
    edge_weight = np.asarray(edge_weight).astype(np.float32)

    def npa(x):
        return np.asarray(x).astype(np.float32)

    emb = npa(params["emb"])
    gnn = params["gnn"]
    ln = params["ln"]
    mlp = params["mlp"]
    out_W = npa(params["out_W"])
    out_b = float(np.asarray(params["out_b"]).reshape(-1)[0])

    src, dst = edge_index[0], edge_index[1]
    w = edge_weight
    w0 = np.where(edge_type == 0, w, 0.0).astype(np.float32)
    w1 = np.where(edge_type == 1, w, 0.0).astype(np.float32)

    K, per_core = _prep_edges(src, dst, w0, w1)
    streams = [_build_core_streams(K, pc) for pc in per_core]
    C = streams[0]["C"]
    NG_b = streams[0]["NG_b"]

    # ---- shared (replicated) parameter tensors
    x0 = emb.astype(np.float16)
    iota2 = np.tile(np.arange(128, dtype=np.float16), (P, 2)).reshape(P, 256)
    wd = np.zeros((P, 9 * 128), np.float16)
    biasw = np.zeros((P, 3 * 128), np.float16)
    for l in range(3):
        Wrel = npa(gnn[l]["Wrel"])           # [2, out, in]
        Wroot = npa(gnn[l]["Wroot"])         # [out, in]
        bb = npa(gnn[l]["b"])                # [out]
        wd[:, l * 384:l * 384 + 128] = Wrel[0].T.astype(np.float16)
        wd[:, l * 384 + 128:l * 384 + 256] = Wrel[1].T.astype(np.float16)
        wd[:, l * 384 + 256:l * 384 + 384] = Wroot.T.astype(np.float16)
        biasw[:, l * 128:(l + 1) * 128] = np.tile(bb, (P, 1)).astype(np.float16)
    lnp = np.zeros((P, 4 * 128), np.float32)
    for l in range(2):
        lnp[:, l * 256:l * 256 + 128] = np.tile(npa(ln[l]["g"]), (P, 1))
        lnp[:, l * 256 + 128:l * 256 + 256] = np.tile(npa(ln[l]["b"]), (P, 1))
    mlpw = np.zeros((P, 864), np.float16)
    mlpb = np.zeros((P, 5), np.float32)
    W1T = npa(mlp[0]["W"]).T                 # [256 in, 256 out]
    mlpw[:, 0:128] = W1T[0:128, 0:128]
    mlpw[:, 128:256] = W1T[0:128, 128:256]
    mlpw[:, 256:384] = W1T[128:256, 0:128]
    mlpw[:, 384:512] = W1T[128:256, 128:256]
    b1 = npa(mlp[0]["b"])
    mlpb[:, 0] = b1[0:128]
    mlpb[:, 1] = b1[128:256]
    W2T = npa(mlp[1]["W"]).T                 # [256, 128]
    mlpw[:, 512:640] = W2T[0:128, :]
    mlpw[:, 640:768] = W2T[128:256, :]
    mlpb[:, 2] = npa(mlp[1]["b"])
    W3T = npa(mlp[2]["W"]).T                 # [128, 64]
    mlpw[:, 768:832] = W3T
    mlpb[0:64, 3] = npa(mlp[2]["b"])
    W4T = npa(mlp[3]["W"]).T                 # [64, 32]
    mlpw[0:64, 832:864] = W4T
    mlpb[0:32, 4] = npa(mlp[3]["b"])
    oww = np.zeros((P, 160), np.float16)
    oww[:, 0:128] = np.tile(out_W[0, 0:128], (P, 1))
    oww[:, 128:160] = np.tile(out_W[0, 128:160], (P, 1))

    # ---- per-core in_maps
    in_maps = []
    BL = BATCH // NCORES
    for c in range(NCORES):
        st = streams[c]
        uvidx = np.zeros((P, 32), np.int32)
        ui = user_indices[c * BL:(c + 1) * BL].astype(np.int32)
        vi = item_indices[c * BL:(c + 1) * BL].astype(np.int32)
        for k in range(16):
            uvidx[:, k] = ui[k * 128:(k + 1) * 128]
            uvidx[:, 16 + k] = vi[k * 128:(k + 1) * 128]
        in_maps.append({
            "x0f": x0, "x0loc": np.concatenate([x0[c * N_LOC:(c + 1) * N_LOC], np.zeros((N_LOC_PAD - N_LOC, EMB), np.float16)]),
            "gidx": st["gidx"], "dstl": st["dstl"], "w01": st["w01"],
            "iota2": iota2, "wd": wd, "biasw": biasw, "lnp": lnp,
            "mlpw": mlpw, "mlpb": mlpb, "oww": oww, "uvidx": uvidx,
        })

    key = ("prog", C, tuple(NG_b), K.tobytes())
    if key not in _cache:
        nc = _build_program(K, NG_b, C)
        _cache[key] = _make_runner(nc, NCORES)
    run = _cache[key]
    results, handle = run(in_maps)
    _cache["last_handle"] = (handle, in_maps, run)

    score = np.concatenate([results[c]["score"] for c in range(NCORES)], axis=0)
    score = score + np.float32(out_b)
    return score.astype(np.float32)
